# revision 1
# baseline (speedup 1.0000x reference)
"""Multi-head attention (B=2, S=2048, D=1024, H=16) on 8 Trainium2 NeuronCores.

Sharding: tensor-parallel over heads x data-parallel over batch.
  core c -> batch g = c // 4, head group r = c % 4 (global heads 4r..4r+3).
Each core computes qkv projections for its 4 heads (w_qkv column slices),
attention for those heads over the full sequence of its batch, then an
AllToAll inside each 4-core group converts the head-sharded attention
output into a sequence-sharded one, so every core runs the full output
projection for its own 512 sequence rows. Host-side work is only
slicing/transposition of inputs and concatenation of outputs.

Device algorithm (per core):
  qT, kT   [256, 2048]  (partition = head*64+hd, free = seq)
  V        [2048, 256]  (partition = seq, free = head*64+hd)
  per (sq-chunk 1024, head pair):
    per sk-tile (128 keys):
      scoresT[sk, sq] = kT.T @ qT   (two K=64 matmuls row-packed in PE)
      pT = exp(0.125 * scoresT)     (ACT, PSUM -> SBUF)
      rowsum_acc += pT              (DVE)
      outT += V.T-slice @ pT        (PSUM accumulate, col-packed pairs)
    rowsum broadcast = ones[128,128].T @ rowsum_acc  (reduce over sk partitions)
    outT *= 1/rowsum  (DVE reciprocal + mul) -> attn_outT [256, 2048]
  Per (pair, sq-chunk): 4-rank AllGather of the finished attn_outT slice
  (first three overlap with remaining attention compute), then each core
  projects its own 512 sequence rows: out = attn_outT.T @ w_proj + b_proj.
"""

import os
import sys

import numpy as np

try:
    import ml_dtypes
    BF16_NP = ml_dtypes.bfloat16
except ImportError:  # pragma: no cover
    BF16_NP = None

for _p in ("/opt/trn_rl_repo",):
    if os.path.isdir(_p) and _p not in sys.path:
        sys.path.append(_p)

import concourse.bass as bass  # noqa: E402
import concourse.mybir as mybir  # noqa: E402
import concourse.tile as tile  # noqa: E402
from concourse import bacc  # noqa: E402
from concourse.bass_utils import run_bass_kernel_spmd  # noqa: E402

B, S, D = 2, 2048, 1024
H, HD = 16, 64
N_CORES = 8
GROUP = 4  # cores per batch group
LH = H // GROUP  # local heads per core = 4
LHD = LH * HD  # 256 local head dims
S_OWN = S // GROUP  # 512 sequence rows owned for the projection
FP32 = mybir.dt.float32
FP32R = mybir.dt.float32r
BF16 = mybir.dt.bfloat16

SQ_CHUNK = 1024  # query-range processed per inner attention pass
N_SQ = S // SQ_CHUNK  # 2
N_SK = S // 128  # 16 key tiles
N_KT = D // 128  # 8 contraction tiles for the projections

_compiled = None
_ONES = np.ones((128, 128), dtype=np.float32)
_ONES16 = None  # set lazily (needs ml_dtypes)


def _build():
    nc = bacc.Bacc(
        "TRN2", target_bir_lowering=False, debug=False, num_devices=N_CORES
    )

    xT_d = nc.dram_tensor("xT", [D, S], BF16, kind="ExternalInput")
    wq_d = nc.dram_tensor("wq", [D, LHD], BF16, kind="ExternalInput")
    wk_d = nc.dram_tensor("wk", [D, LHD], BF16, kind="ExternalInput")
    wv_d = nc.dram_tensor("wv", [D, LHD], BF16, kind="ExternalInput")
    wp_d = nc.dram_tensor("wp", [D, D], BF16, kind="ExternalInput")
    bq_d = nc.dram_tensor("bq", [LHD, 1], FP32, kind="ExternalInput")
    bk_d = nc.dram_tensor("bk", [LHD, 1], FP32, kind="ExternalInput")
    bv_d = nc.dram_tensor("bv", [128, LHD], FP32, kind="ExternalInput")
    bp_d = nc.dram_tensor("bp", [128, D], FP32, kind="ExternalInput")
    ones_d = nc.dram_tensor("ones", [128, 128], FP32R, kind="ExternalInput")
    ones16_d = nc.dram_tensor("ones16", [128, 8], BF16, kind="ExternalInput")
    out_d = nc.dram_tensor("out", [S_OWN, D], FP32, kind="ExternalOutput")

    # Two AllGathers (one per head pair) inside each 4-core batch group;
    # the first runs while the second pair's attention still computes.
    ag_in = [[nc.dram_tensor(f"ag_in{p}_{c}", [128, SQ_CHUNK], BF16)
              for c in range(N_SQ)] for p in range(2)]
    ag_out = [nc.dram_tensor(f"ag_out{p}", [N_SQ * GROUP * 128, SQ_CHUNK], BF16)
              for p in range(2)]
    groups = [[0, 1, 2, 3], [4, 5, 6, 7]]

    with tile.TileContext(nc) as tc:
        import contextlib

        with contextlib.ExitStack() as stk:
            # ---- long-lived pools -------------------------------------
            qk_pool = stk.enter_context(tc.tile_pool(name="qk", bufs=1))
            v_pool = stk.enter_context(tc.tile_pool(name="v", bufs=1))
            ao_pool = stk.enter_context(tc.tile_pool(name="ao", bufs=1))
            const_pool = stk.enter_context(tc.tile_pool(name="const", bufs=1))
            wp_pool = stk.enter_context(tc.tile_pool(name="wp", bufs=1))

            qT = [qk_pool.tile([128, S], BF16, name=f"qT{j}", tag=f"qT{j}") for j in range(2)]
            kT = [qk_pool.tile([128, S], BF16, name=f"kT{j}", tag=f"kT{j}") for j in range(2)]
            vp = [v_pool.tile([128, LH * 65], BF16, name=f"v{m}", tag=f"v{m}") for m in range(16)]
            aoT = [ao_pool.tile([128, S], BF16, name=f"ao{p}", tag=f"ao{p}") for p in range(2)]

            ones_t = const_pool.tile([128, 128], FP32R, tag="ones")
            nc.sync.dma_start(ones_t[:], ones_d.ap())
            bq_t = [const_pool.tile([128, 1], FP32, name=f"bq{j}", tag=f"bq{j}")
                    for j in range(2)]
            bk_t = [const_pool.tile([128, 1], FP32, name=f"bk{j}", tag=f"bk{j}")
                    for j in range(2)]
            bv_t = const_pool.tile([128, LHD], FP32, tag="bv")
            bp_t = const_pool.tile([128, D], FP32, tag="bp")
            for j in range(2):
                jsl = slice(j * 128, (j + 1) * 128)
                nc.sync.dma_start(bq_t[j][:], bq_d.ap()[jsl, :])
                nc.sync.dma_start(bk_t[j][:], bk_d.ap()[jsl, :])
            nc.sync.dma_start(bv_t[:], bv_d.ap())
            nc.sync.dma_start(bp_t[:], bp_d.ap())

            wp_t = [wp_pool.tile([128, D], BF16, name=f"wp{k}", tag=f"wp{k}") for k in range(N_KT)]

            # ---- PE warm-up: keep the HAM clock gate open during the
            # input DMA ramp (dummy matmuls into a scratch psum bank)
            with tc.tile_pool(name="warm", bufs=1, space="PSUM") as warm_pool:
                wps = warm_pool.tile([128, 128], FP32, tag="warm")
                for w in range(32):
                    nc.tensor.matmul(
                        wps[:],
                        ones_t[:],
                        ones_t[:],
                        start=True, stop=True,
                        skip_group_check=True,
                    )

            # ---- phase A: qkv projections -----------------------------
            with (
                tc.tile_pool(name="x", bufs=1) as x_pool,
                tc.tile_pool(name="w", bufs=1) as w_pool,
                tc.tile_pool(name="psA", bufs=1, space="PSUM") as psA,
            ):
                x_t = [x_pool.tile([128, S], BF16, name=f"x{k}", tag=f"x{k}") for k in range(N_KT)]
                wq_t = [w_pool.tile([128, LHD], BF16, name=f"wq{k}", tag=f"wq{k}") for k in range(N_KT)]
                wk_t = [w_pool.tile([128, LHD], BF16, name=f"wk{k}", tag=f"wk{k}") for k in range(N_KT)]
                wv_t = [w_pool.tile([128, LHD], BF16, name=f"wv{k}", tag=f"wv{k}") for k in range(N_KT)]
                # interleave x/w loads k-major and fan out across four DMA
                # queues so the first contraction tiles land quickly
                dma_engs = [nc.sync, nc.scalar]
                for k in range(N_KT):
                    sl = slice(k * 128, (k + 1) * 128)
                    eng = dma_engs[k % 2]
                    eng.dma_start(x_t[k][:], xT_d.ap()[sl, :])
                    eng2 = dma_engs[(k + 1) % 2]
                    eng2.dma_start(wq_t[k][:], wq_d.ap()[sl, :])
                    eng2.dma_start(wk_t[k][:], wk_d.ap()[sl, :])
                    eng2.dma_start(wv_t[k][:], wv_d.ap()[sl, :])

                # qT / kT / V in PSUM waves, contraction k outermost so the
                # PE follows the xT tiles as they stream in from HBM
                for j in range(2):
                    ps_q = [psA.tile([128, 512], FP32, name=f"psq{j}{sc}", tag=f"psA{sc}") for sc in range(4)]
                    ps_k = [psA.tile([128, 512], FP32, name=f"psk{j}{sc}", tag=f"psA{sc+4}") for sc in range(4)]
                    for k in range(N_KT):
                        for sc in range(4):
                            ssl = slice(sc * 512, (sc + 1) * 512)
                            nc.tensor.matmul(
                                ps_q[sc][:],
                                wq_t[k][:, j * 128 : (j + 1) * 128],
                                x_t[k][:, ssl],
                                start=(k == 0), stop=(k == N_KT - 1),
                            )
                            nc.tensor.matmul(
                                ps_k[sc][:],
                                wk_t[k][:, j * 128 : (j + 1) * 128],
                                x_t[k][:, ssl],
                                start=(k == 0), stop=(k == N_KT - 1),
                            )
                    for sc in range(4):
                        ssl = slice(sc * 512, (sc + 1) * 512)
                        nc.vector.tensor_scalar(
                            qT[j][:, ssl], ps_q[sc][:], bq_t[j][:], None,
                            mybir.AluOpType.add,
                        )
                        nc.vector.tensor_scalar(
                            kT[j][:, ssl], ps_k[sc][:], bk_t[j][:], None,
                            mybir.AluOpType.add,
                        )
                # V: [s-tile 128, 256] = x.T @ wv, two waves of 8 m-tiles
                for wave in range(2):
                    ps_v = [psA.tile([128, LHD], FP32, name=f"psv{wave}{i}", tag=f"psA{i}") for i in range(8)]
                    for k in range(N_KT):
                        for i in range(8):
                            m = wave * 8 + i
                            nc.tensor.matmul(
                                ps_v[i][:],
                                x_t[k][:, m * 128 : (m + 1) * 128],
                                wv_t[k][:],
                                start=(k == 0), stop=(k == N_KT - 1),
                            )
                    for i in range(8):
                        m = wave * 8 + i
                        for h in range(LH):
                            nc.vector.tensor_tensor(
                                vp[m][:, 65 * h : 65 * h + 64],
                                ps_v[i][:, 64 * h : 64 * h + 64],
                                bv_t[:, 64 * h : 64 * h + 64],
                                mybir.AluOpType.add,
                            )
                        nc.sync.dma_start(vp[m][:, 64::65], ones16_d.ap()[:, 0:LH])

            # weight prefetch for phase D (scheduler fills DMA gaps)
            for k in range(N_KT):
                nc.sync.dma_start(wp_t[k][:], wp_d.ap()[k * 128 : (k + 1) * 128, :])

            at_pool = stk.enter_context(tc.tile_pool(name="at", bufs=1))
            at_t = [at_pool.tile([128, S_OWN], BF16, name=f"at{k}", tag=f"at{k}")
                    for k in range(N_KT)]
            pid = nc.gpsimd.partition_id()
            rank = pid % GROUP
            col0 = (rank % 2) * S_OWN

            def load_at(k):
                row0 = (rank // 2) * 512 + 128 * (k // 2)
                nc.gpsimd.dma_start(
                    at_t[k][:],
                    ag_out[k % 2].ap()[bass.ds(row0, 128), bass.ds(col0, S_OWN)],
                )

            # ---- phase B: attention -----------------------------------
            with (
                tc.tile_pool(name="p", bufs=4) as p_pool,
                tc.tile_pool(name="rr", bufs=4) as rr_pool,
                tc.tile_pool(name="rcp", bufs=2) as rcp_pool,
                tc.tile_pool(name="psc", bufs=2, space="PSUM") as ps_sc,
                tc.tile_pool(name="pacc", bufs=1, space="PSUM") as ps_acc,
            ):
                for p in range(2):  # head pair: local heads 2p, 2p+1
                    for cq in range(N_SQ):
                        qsl = slice(cq * SQ_CHUNK, (cq + 1) * SQ_CHUNK)
                        # row 64 of each acc collects the softmax denominator
                        # via the ones column appended to V
                        acc_a = ps_acc.tile([65, SQ_CHUNK], FP32, tag="acca")
                        acc_b = ps_acc.tile([65, SQ_CHUNK], FP32, tag="accb")
                        for t in range(N_SK):
                            tsl = slice(t * 128, (t + 1) * 128)
                            sca = ps_sc.tile([128, SQ_CHUNK], FP32, tag="sc")
                            scb = ps_sc.tile([128, SQ_CHUNK], FP32, tag="sc")
                            for u in range(SQ_CHUNK // 512):
                                usl = slice(u * 512, (u + 1) * 512)
                                gsl = slice(cq * SQ_CHUNK + u * 512,
                                            cq * SQ_CHUNK + (u + 1) * 512)
                                nc.tensor.matmul(
                                    sca[:, usl],
                                    kT[p][0:64, tsl],
                                    qT[p][0:64, gsl],
                                    start=True, stop=True,
                                    tile_position=(0, 0),
                                )
                                nc.tensor.matmul(
                                    scb[:, usl],
                                    kT[p][64:128, tsl],
                                    qT[p][64:128, gsl],
                                    start=True, stop=True,
                                    tile_position=(64, 0),
                                )
                            pa = p_pool.tile([128, SQ_CHUNK], BF16, tag="pt")
                            pb = p_pool.tile([128, SQ_CHUNK], BF16, tag="pt")
                            nc.scalar.activation(
                                pa[:], sca[:],
                                mybir.ActivationFunctionType.Exp, scale=0.125,
                            )
                            nc.scalar.activation(
                                pb[:], scb[:],
                                mybir.ActivationFunctionType.Exp, scale=0.125,
                            )
                            for u in range(SQ_CHUNK // 512):
                                usl = slice(u * 512, (u + 1) * 512)
                                nc.tensor.matmul(
                                    acc_a[:, usl],
                                    vp[t][:, 65 * (2 * p) : 65 * (2 * p) + 65],
                                    pa[:, usl],
                                    start=(t == 0), stop=(t == N_SK - 1),
                                )
                                nc.tensor.matmul(
                                    acc_b[:, usl],
                                    vp[t][:, 65 * (2 * p + 1) : 65 * (2 * p + 1) + 65],
                                    pb[:, usl],
                                    start=(t == 0), stop=(t == N_SK - 1),
                                )
                        # normalize: 1/rowsum broadcast across the 64 head dims
                        for acc, half in ((acc_a, 0), (acc_b, 1)):
                            rrow = rr_pool.tile([1, SQ_CHUNK], FP32R, tag="rrow")
                            nc.vector.tensor_copy(rrow[:], acc[64:65, :])
                            rb = ps_sc.tile([64, SQ_CHUNK], FP32, tag="sc")
                            for u in range(SQ_CHUNK // 512):
                                usl = slice(u * 512, (u + 1) * 512)
                                nc.tensor.matmul(
                                    rb[:, usl], ones_t[0:1, 0:64], rrow[:, usl],
                                    start=True, stop=True,
                                )
                            rc = rcp_pool.tile([64, SQ_CHUNK], FP32, tag="rc")
                            nc.vector.reciprocal_approx_fast(rc[:], rb[:])
                            nc.vector.tensor_tensor(
                                aoT[p][64 * half : 64 * half + 64, qsl],
                                acc[0:64, :], rc[:],
                                mybir.AluOpType.mult,
                            )
                        # gather this (pair, sq-chunk) while compute continues
                        nc.sync.dma_start(ag_in[p][cq].ap(), aoT[p][:, qsl])
                        nc.gpsimd.collective_compute(
                            "AllGather",
                            mybir.AluOpType.bypass,
                            replica_groups=groups,
                            ins=[ag_in[p][cq].ap()],
                            outs=[ag_out[p].ap()[cq * 512 : (cq + 1) * 512, :]],
                        )
                        if p == 1:
                            # pair-0 loads go after pair-1's first collective
                            # trigger (their wait is then already satisfied and
                            # cannot stall the queue); pair-1 loads at the end
                            for k in range(cq, N_KT, 2):
                                load_at(k)


            # ---- phase D: output projection on own 512 rows -----------
            with (
                tc.tile_pool(name="outp", bufs=4) as out_pool,
                tc.tile_pool(name="psD", bufs=1, space="PSUM") as psD,
            ):
                # logical head-row block k lives in ag_out[k%2]; the
                # gathered rows are stacked [sq-chunk][group-rank][128],
                # and this core's sequence window picks chunk (rank//2)
                # at column offset (rank%2)*512
                # two-pass projection: every tile's even-k contributions
                # (available after the early pair-0 gathers) run first and can
                # overlap the final AllGather; the odd-k half follows
                tiles = [(m, nb) for m in range(S_OWN // 128) for nb in range(2)]
                ps_all = {}
                for m, nb in tiles:
                    ps_all[(m, nb)] = psD.tile(
                        [128, 512], FP32, name=f"psD{m}{nb}", tag=f"psD{m}{nb}"
                    )
                for ks, first, last in (((0, 2, 4, 6), True, False),
                                        ((1, 3, 5, 7), False, True)):
                    for m, nb in tiles:
                        msl = slice(m * 128, (m + 1) * 128)
                        nsl = slice(nb * 512, (nb + 1) * 512)
                        for ki, k in enumerate(ks):
                            nc.tensor.matmul(
                                ps_all[(m, nb)][:],
                                at_t[k][:, msl],
                                wp_t[k][:, nsl],
                                start=(first and ki == 0),
                                stop=(last and ki == 3),
                            )
                for m, nb in tiles:
                    msl = slice(m * 128, (m + 1) * 128)
                    nsl = slice(nb * 512, (nb + 1) * 512)
                    ot = out_pool.tile([128, 512], FP32, tag="ot")
                    nc.vector.tensor_tensor(
                        ot[:], ps_all[(m, nb)][:], bp_t[:, nsl], mybir.AluOpType.add
                    )
                    nc.sync.dma_start(out_d.ap()[msl, nsl], ot[:])

    nc.compile()
    return nc


def _get_program():
    global _compiled
    if _compiled is None:
        _compiled = _build()
    return _compiled


def _make_in_maps(x, w_qkv, b_qkv, w_proj, b_proj):
    x = np.asarray(x, dtype=np.float32)
    w_qkv = np.asarray(w_qkv, dtype=np.float32)
    b_qkv = np.asarray(b_qkv, dtype=np.float32)
    w_proj = np.asarray(w_proj, dtype=np.float32)
    b_proj = np.asarray(b_proj, dtype=np.float32)

    global _ONES16
    if _ONES16 is None:
        _ONES16 = np.ones((128, 8), dtype=BF16_NP)
    wp16 = w_proj.astype(BF16_NP)
    bp_b = np.ascontiguousarray(np.broadcast_to(b_proj.reshape(1, D), (128, D)))
    in_maps = []
    for c in range(N_CORES):
        g, r = c // GROUP, c % GROUP
        xT = np.ascontiguousarray(x[g].T)
        in_maps.append(
            {
                "xT": xT.astype(BF16_NP),
                "wq": w_qkv[:, 0 * D + r * LHD : 0 * D + (r + 1) * LHD].astype(BF16_NP),
                "wk": w_qkv[:, 1 * D + r * LHD : 1 * D + (r + 1) * LHD].astype(BF16_NP),
                "wv": w_qkv[:, 2 * D + r * LHD : 2 * D + (r + 1) * LHD].astype(BF16_NP),
                "wp": wp16,
                "bq": np.ascontiguousarray(b_qkv[0 * D + r * LHD : 0 * D + (r + 1) * LHD].reshape(LHD, 1)),
                "bk": np.ascontiguousarray(b_qkv[1 * D + r * LHD : 1 * D + (r + 1) * LHD].reshape(LHD, 1)),
                "bv": np.ascontiguousarray(
                    np.broadcast_to(
                        b_qkv[2 * D + r * LHD : 2 * D + (r + 1) * LHD].reshape(1, LHD),
                        (128, LHD),
                    )
                ),
                "bp": bp_b,
                "ones": _ONES,
                "ones16": _ONES16,
            }
        )
    return in_maps


def _assemble(results):
    out = np.empty((B, S, D), dtype=np.float32)
    for c in range(N_CORES):
        g, r = c // GROUP, c % GROUP
        out[g, r * S_OWN : (r + 1) * S_OWN, :] = results[c]["out"]
    return out


def kernel(x, w_qkv, b_qkv, w_proj, b_proj):
    nc = _get_program()
    in_maps = _make_in_maps(x, w_qkv, b_qkv, w_proj, b_proj)
    res = run_bass_kernel_spmd(nc, in_maps, list(range(N_CORES)))
    return _assemble(res.results)



# revision 3
# speedup vs baseline: 1.4503x; 1.4503x over previous
"""Multi-head attention (B=2, S=2048, D=1024, H=16) on 8 Trainium2 NeuronCores.

Sharding: tensor-parallel over heads x data-parallel over batch.
  core c -> batch g = c // 4, head group r = c % 4 (global heads 4r..4r+3).
Each core computes qkv for its 4 heads (two head pairs), attention over the
full sequence of its batch, and a PARTIAL output projection over its own 256
head-dims for ALL 2048 rows.  The four partials per batch are summed on the
host (plus b_proj) -- no device collectives at all.

Device schedule (emission order == per-engine program order):
  phase A: stream xT in k-tiles, qk(pair0) k-major into 8 PSUM banks,
           then v(pair0) m-major.
  phase B: for pair p, for cq (512-query chunk), for t (128-key tile):
             scores: two row-packed K=64 matmuls (head 2p rows 0-63,
                     head 2p+1 rows 64-127) into one [128,1024] PSUM tile
                     (two banks, no write conflict),
             ONE exp ACT over [128,1024] (both heads) -> bf16,
             two AV matmuls accumulating [65,512] (ones column of V picks
                     up the softmax denominator).
           qkv(pair1) is emitted in small pieces between t-iterations of
           pair0's attention; projection chunks likewise ride inside
           pair1's attention.  The scalar engine (exp) is the critical
           resource; the PE fills its shadow.
  normalize per (p,cq): rowsum bcast via tiny K=1 matmul, reciprocal and
           multiply on DVE -> aoT bf16.
  proj: per s-tile: psum[128,512] = aoT[0].T @ wp[0] + aoT[1].T @ wp[1],
           copied to fp32 and DMA'd out (partial, host sums).
"""

import os
import sys

import numpy as np

try:
    import ml_dtypes
    BF16_NP = ml_dtypes.bfloat16
except ImportError:  # pragma: no cover
    BF16_NP = None

for _p in ("/opt/trn_rl_repo",):
    if os.path.isdir(_p) and _p not in sys.path:
        sys.path.append(_p)

import concourse.bass as bass  # noqa: E402
import concourse.mybir as mybir  # noqa: E402
import concourse.tile as tile  # noqa: E402
from concourse import bacc  # noqa: E402
from concourse.bass_utils import run_bass_kernel_spmd  # noqa: E402

B, S, D = 2, 2048, 1024
H, HD = 16, 64
N_CORES = 8
GROUP = 4           # cores per batch group
LH = H // GROUP     # local heads per core = 4 (2 pairs)
LHD = LH * HD       # 256 local head dims
FP32 = mybir.dt.float32
FP32R = mybir.dt.float32r
BF16 = mybir.dt.bfloat16

SQ = 512            # query chunk
N_CQ = S // SQ      # 4
N_SK = S // 128     # 16 key tiles
N_KT = D // 128     # 8 contraction tiles

_compiled = None
_ONES = np.ones((1, 64), dtype=np.float32)


def _build():
    nc = bacc.Bacc(
        "TRN2", target_bir_lowering=False, debug=False, num_devices=N_CORES
    )

    xT_d = nc.dram_tensor("xT", [D, S], BF16, kind="ExternalInput")
    wq_d = nc.dram_tensor("wq", [D, LHD], BF16, kind="ExternalInput")
    wk_d = nc.dram_tensor("wk", [D, LHD], BF16, kind="ExternalInput")
    wv_d = nc.dram_tensor("wv", [D, LHD], BF16, kind="ExternalInput")
    wp_d = nc.dram_tensor("wp", [LHD, D], BF16, kind="ExternalInput")
    bq_d = nc.dram_tensor("bq", [LHD, 1], FP32, kind="ExternalInput")
    bk_d = nc.dram_tensor("bk", [LHD, 1], FP32, kind="ExternalInput")
    bv_d = nc.dram_tensor("bv", [128, LHD], FP32, kind="ExternalInput")
    ones_d = nc.dram_tensor("ones", [1, 64], FP32R, kind="ExternalInput")
    out_d = nc.dram_tensor("out", [S, D], FP32, kind="ExternalOutput")

    with tile.TileContext(nc) as tc:
        import contextlib

        with contextlib.ExitStack() as stk:
            # ---- long-lived SBUF pools --------------------------------
            qk_pool = stk.enter_context(tc.tile_pool(name="qk", bufs=1))
            v_pool = stk.enter_context(tc.tile_pool(name="v", bufs=1))
            ao_pool = stk.enter_context(tc.tile_pool(name="ao", bufs=1))
            const_pool = stk.enter_context(tc.tile_pool(name="const", bufs=1))
            w_pool = stk.enter_context(tc.tile_pool(name="w", bufs=1))
            x_pool = stk.enter_context(tc.tile_pool(name="x", bufs=1))

            qT = [qk_pool.tile([128, S], BF16, name=f"qT{p}", tag=f"qT{p}")
                  for p in range(2)]
            kT = [qk_pool.tile([128, S], BF16, name=f"kT{p}", tag=f"kT{p}")
                  for p in range(2)]
            # vp[p][m]: [128 keys, 130] = head2p v | 1.0 | head2p+1 v | 1.0
            vp = [[v_pool.tile([128, 130], BF16, name=f"v{p}_{m}",
                               tag=f"v{p}_{m}") for m in range(N_SK)]
                  for p in range(2)]
            aoT = [ao_pool.tile([128, S], BF16, name=f"ao{p}", tag=f"ao{p}")
                   for p in range(2)]

            ones_t = const_pool.tile([1, 64], FP32R, tag="ones")
            nc.sync.dma_start(ones_t[:], ones_d.ap())
            bq_t = [const_pool.tile([128, 1], FP32, name=f"bq{p}",
                                    tag=f"bq{p}") for p in range(2)]
            bk_t = [const_pool.tile([128, 1], FP32, name=f"bk{p}",
                                    tag=f"bk{p}") for p in range(2)]
            bv_t = const_pool.tile([128, LHD], FP32, tag="bv")
            for p in range(2):
                psl = slice(p * 128, (p + 1) * 128)
                nc.sync.dma_start(bq_t[p][:], bq_d.ap()[psl, :])
                nc.sync.dma_start(bk_t[p][:], bk_d.ap()[psl, :])
            nc.sync.dma_start(bv_t[:], bv_d.ap())

            x_t = [x_pool.tile([128, S], BF16, name=f"x{k}", tag=f"x{k}")
                   for k in range(N_KT)]
            wq_t = [w_pool.tile([128, LHD], BF16, name=f"wq{k}", tag=f"wq{k}")
                    for k in range(N_KT)]
            wk_t = [w_pool.tile([128, LHD], BF16, name=f"wk{k}", tag=f"wk{k}")
                    for k in range(N_KT)]
            wv_t = [w_pool.tile([128, LHD], BF16, name=f"wv{k}", tag=f"wv{k}")
                    for k in range(N_KT)]
            wp_t = [w_pool.tile([128, D], BF16, name=f"wp{p}", tag=f"wp{p}")
                    for p in range(2)]

            # input DMA: x on sync queue, weights on scalar queue (scalar
            # engine is idle during phase A)
            for k in range(N_KT):
                sl = slice(k * 128, (k + 1) * 128)
                nc.sync.dma_start(x_t[k][:], xT_d.ap()[sl, :])
                nc.scalar.dma_start(wq_t[k][:], wq_d.ap()[sl, :])
                nc.scalar.dma_start(wk_t[k][:], wk_d.ap()[sl, :])
                nc.scalar.dma_start(wv_t[k][:], wv_d.ap()[sl, :])
            for p in range(2):
                nc.scalar.dma_start(wp_t[p][:],
                                    wp_d.ap()[p * 128:(p + 1) * 128, :])

            # ---- phase A: qk(pair0) k-major, v(pair0) m-major ---------
            with tc.tile_pool(name="psA", bufs=1, space="PSUM") as psA:
                ps_q = [psA.tile([128, SQ], FP32, name=f"psq{sc}",
                                 tag=f"psA{sc}") for sc in range(4)]
                ps_k = [psA.tile([128, SQ], FP32, name=f"psk{sc}",
                                 tag=f"psA{sc + 4}") for sc in range(4)]
                for k in range(N_KT):
                    for sc in range(4):
                        ssl = slice(sc * SQ, (sc + 1) * SQ)
                        nc.tensor.matmul(
                            ps_q[sc][:], wq_t[k][:, 0:128], x_t[k][:, ssl],
                            start=(k == 0), stop=(k == N_KT - 1),
                        )
                        nc.tensor.matmul(
                            ps_k[sc][:], wk_t[k][:, 0:128], x_t[k][:, ssl],
                            start=(k == 0), stop=(k == N_KT - 1),
                        )
                for sc in range(4):
                    ssl = slice(sc * SQ, (sc + 1) * SQ)
                    nc.vector.tensor_scalar(
                        qT[0][:, ssl], ps_q[sc][:], bq_t[0][:], None,
                        mybir.AluOpType.add,
                    )
                    nc.vector.tensor_scalar(
                        kT[0][:, ssl], ps_k[sc][:], bk_t[0][:], None,
                        mybir.AluOpType.add,
                    )

            def drain_v(p, m, ps):
                nc.vector.tensor_tensor(
                    vp[p][m][:, 0:64], ps[:, 0:64], bv_t[:, p * 128:p * 128 + 64],
                    mybir.AluOpType.add,
                )
                nc.vector.tensor_tensor(
                    vp[p][m][:, 65:129], ps[:, 64:128],
                    bv_t[:, p * 128 + 64:p * 128 + 128],
                    mybir.AluOpType.add,
                )
                nc.vector.memset(vp[p][m][:, 64::65], 1.0)

            with tc.tile_pool(name="psV", bufs=4, space="PSUM") as psV:
                for m in range(N_SK):
                    ps = psV.tile([128, 128], FP32, tag="v")
                    for k in range(N_KT):
                        nc.tensor.matmul(
                            ps[:], x_t[k][:, m * 128:(m + 1) * 128],
                            wv_t[k][:, 0:128],
                            start=(k == 0), stop=(k == N_KT - 1),
                        )
                    drain_v(0, m, ps)

            # ---- phase B pools ----------------------------------------
            sc_pool = stk.enter_context(
                tc.tile_pool(name="sc", bufs=2, space="PSUM"))
            acc_pool = stk.enter_context(
                tc.tile_pool(name="acc", bufs=1, space="PSUM"))
            misc_pool = stk.enter_context(
                tc.tile_pool(name="misc", bufs=2, space="PSUM"))
            p_pool = stk.enter_context(tc.tile_pool(name="pt", bufs=4))
            rr_pool = stk.enter_context(tc.tile_pool(name="rr", bufs=2))
            rc_pool = stk.enter_context(tc.tile_pool(name="rc", bufs=2))
            ost_pool = stk.enter_context(tc.tile_pool(name="ost", bufs=2))

            # ---- deferred emission units (PE filler work) -------------
            filler = []

            def qk1_chunk_parts(which, sc):
                # q/k pair-1 chunk: 8 accumulating MMs + DVE drain, split
                # into 2-MM pieces so injection granularity stays ~0.5us
                w_t, b_t, dstT = ((wq_t, bq_t[1], qT[1]) if which == "q"
                                  else (wk_t, bk_t[1], kT[1]))
                ssl = slice(sc * SQ, (sc + 1) * SQ)
                state = {}

                def piece(k0):
                    def emit():
                        if k0 == 0:
                            state["ps"] = misc_pool.tile(
                                [128, SQ], FP32,
                                name=f"mqk{which}{sc}", tag="m")
                        ps = state["ps"]
                        for k in (k0, k0 + 1):
                            nc.tensor.matmul(
                                ps[:], w_t[k][:, 128:256], x_t[k][:, ssl],
                                start=(k == 0), stop=(k == N_KT - 1),
                            )
                        if k0 == N_KT - 2:
                            nc.vector.tensor_scalar(
                                dstT[:, ssl], ps[:], b_t[:], None,
                                mybir.AluOpType.add,
                            )
                    return emit
                return [piece(k0) for k0 in range(0, N_KT, 2)]

            def v1_chunk_parts(m):
                state = {}

                def piece(k0):
                    def emit():
                        if k0 == 0:
                            state["ps"] = misc_pool.tile(
                                [128, SQ], FP32, name=f"mv{m}", tag="m")
                        ps = state["ps"]
                        for k in (k0, k0 + 1):
                            nc.tensor.matmul(
                                ps[:, 0:128],
                                x_t[k][:, m * 128:(m + 1) * 128],
                                wv_t[k][:, 128:256],
                                start=(k == 0), stop=(k == N_KT - 1),
                            )
                        if k0 == N_KT - 2:
                            drain_v(1, m, ps[:, 0:128])
                    return emit
                return [piece(k0) for k0 in range(0, N_KT, 2)]

            for sc in range(4):
                filler.extend(qk1_chunk_parts("q", sc))
                filler.extend(qk1_chunk_parts("k", sc))
            for m in range(N_SK):
                filler.extend(v1_chunk_parts(m))

            def proj_unit(m, nb):
                # partial projection for s-tile m, dout half nb
                msl = slice(m * 128, (m + 1) * 128)
                nsl = slice(nb * SQ, (nb + 1) * SQ)

                def emit():
                    ps = misc_pool.tile([128, SQ], FP32,
                                        name=f"mpj{m}_{nb}", tag="m")

                    for p in range(2):
                        nc.tensor.matmul(
                            ps[:], aoT[p][:, msl], wp_t[p][:, nsl],
                            start=(p == 0), stop=(p == 1),
                        )
                    ot = ost_pool.tile([128, SQ], FP32,
                                       name=f"ot{m}_{nb}", tag="ot")
                    nc.vector.tensor_copy(ot[:], ps[:])
                    nc.gpsimd.dma_start(out_d.ap()[msl, nsl], ot[:])
                return emit

            def inject(n):
                for _ in range(n):
                    if filler:
                        filler.pop(0)()

            # ---- phase B: attention -----------------------------------
            for p in range(2):
                for cq in range(N_CQ):
                    qsl = slice(cq * SQ, (cq + 1) * SQ)
                    acc_a = acc_pool.tile([65, SQ], FP32, tag="acca")
                    acc_b = acc_pool.tile([65, SQ], FP32, tag="accb")
                    for t in range(N_SK):
                        tsl = slice(t * 128, (t + 1) * 128)
                        sc_ab = sc_pool.tile([128, 2 * SQ], FP32, tag="sc")
                        nc.tensor.matmul(
                            sc_ab[:, 0:SQ], kT[p][0:64, tsl],
                            qT[p][0:64, qsl],
                            start=True, stop=True, tile_position=(0, 0),
                        )
                        nc.tensor.matmul(
                            sc_ab[:, SQ:2 * SQ], kT[p][64:128, tsl],
                            qT[p][64:128, qsl],
                            start=True, stop=True, tile_position=(64, 0),
                        )
                        pab = p_pool.tile([128, 2 * SQ], BF16, tag="pt")
                        nc.scalar.activation(
                            pab[:], sc_ab[:],
                            mybir.ActivationFunctionType.Exp, scale=0.125,
                        )
                        nc.tensor.matmul(
                            acc_a[:], vp[p][t][:, 0:65], pab[:, 0:SQ],
                            start=(t == 0), stop=(t == N_SK - 1),
                        )
                        nc.tensor.matmul(
                            acc_b[:], vp[p][t][:, 65:130], pab[:, SQ:2 * SQ],
                            start=(t == 0), stop=(t == N_SK - 1),
                        )
                        inject(2 if p == 0 else 1)
                    # normalize both heads of this (p, cq)
                    for acc, half in ((acc_a, 0), (acc_b, 1)):
                        rrow = rr_pool.tile([1, SQ], FP32R, tag="rrow")
                        nc.vector.tensor_copy(rrow[:], acc[64:65, :])
                        rbt = misc_pool.tile([128, SQ], FP32, tag="m")
                        nc.tensor.matmul(
                            rbt[0:64, :], ones_t[0:1, 0:64], rrow[:],
                            start=True, stop=True,
                        )
                        rc = rc_pool.tile([64, SQ], FP32, tag="rc")
                        nc.vector.reciprocal_approx_fast(rc[:], rbt[0:64, :])
                        nc.vector.tensor_tensor(
                            aoT[p][64 * half:64 * half + 64, qsl],
                            acc[0:64, :], rc[:],
                            mybir.AluOpType.mult,
                        )
                    if p == 1:
                        # projection for the chunk finished one cq ago
                        # rides inside the next chunk's t-loop via filler
                        for m in range(cq * 4, cq * 4 + 4):
                            for nb in range(2):
                                filler.append(proj_unit(m, nb))
            # flush remaining filler (last chunk's projection)
            while filler:
                filler.pop(0)()

    nc.compile()
    return nc


def _get_program():
    global _compiled
    if _compiled is None:
        _compiled = _build()
    return _compiled


def _make_in_maps(x, w_qkv, b_qkv, w_proj, b_proj):
    x = np.asarray(x, dtype=np.float32)
    w_qkv = np.asarray(w_qkv, dtype=np.float32)
    b_qkv = np.asarray(b_qkv, dtype=np.float32)
    w_proj = np.asarray(w_proj, dtype=np.float32)

    in_maps = []
    for c in range(N_CORES):
        g, r = c // GROUP, c % GROUP
        xT = np.ascontiguousarray(x[g].T)
        csl = slice(r * LHD, (r + 1) * LHD)
        in_maps.append(
            {
                "xT": xT.astype(BF16_NP),
                "wq": w_qkv[:, 0 * D + r * LHD:0 * D + (r + 1) * LHD].astype(BF16_NP),
                "wk": w_qkv[:, 1 * D + r * LHD:1 * D + (r + 1) * LHD].astype(BF16_NP),
                "wv": w_qkv[:, 2 * D + r * LHD:2 * D + (r + 1) * LHD].astype(BF16_NP),
                "wp": np.ascontiguousarray(w_proj[csl, :]).astype(BF16_NP),
                "bq": np.ascontiguousarray(
                    b_qkv[0 * D + r * LHD:0 * D + (r + 1) * LHD].reshape(LHD, 1)),
                "bk": np.ascontiguousarray(
                    b_qkv[1 * D + r * LHD:1 * D + (r + 1) * LHD].reshape(LHD, 1)),
                "bv": np.ascontiguousarray(
                    np.broadcast_to(
                        b_qkv[2 * D + r * LHD:2 * D + (r + 1) * LHD].reshape(1, LHD),
                        (128, LHD),
                    )
                ),
                "ones": _ONES,
            }
        )
    return in_maps


def _assemble(results, b_proj):
    out = np.empty((B, S, D), dtype=np.float32)
    for g in range(B):
        acc = results[g * GROUP]["out"].copy()
        for r in range(1, GROUP):
            acc += results[g * GROUP + r]["out"]
        out[g] = acc + np.asarray(b_proj, dtype=np.float32).reshape(1, D)
    return out


def kernel(x, w_qkv, b_qkv, w_proj, b_proj):
    nc = _get_program()
    in_maps = _make_in_maps(x, w_qkv, b_qkv, w_proj, b_proj)
    res = run_bass_kernel_spmd(nc, in_maps, list(range(N_CORES)))
    return _assemble(res.results, b_proj)


# revision 5
# speedup vs baseline: 1.4548x; 1.0031x over previous
"""Multi-head attention (B=2, S=2048, D=1024, H=16) on 8 Trainium2 NeuronCores.

Sharding: tensor-parallel over heads x data-parallel over batch.
  core c -> batch g = c // 4, head group r = c % 4 (global heads 4r..4r+3).
Each core computes qkv for its 4 heads (two head pairs), attention over the
full sequence of its batch, and a PARTIAL output projection over its own 256
head-dims for ALL 2048 rows.  The four partials per batch are summed on the
host (plus b_proj) -- no device collectives at all.

Device schedule (emission order == per-engine program order):
  phase A: stream xT in k-tiles, qk(pair0) k-major into 8 PSUM banks,
           then v(pair0) m-major.
  phase B: for pair p, for cq (512-query chunk), for t (128-key tile):
             scores: two row-packed K=64 matmuls (head 2p rows 0-63,
                     head 2p+1 rows 64-127) into one [128,1024] PSUM tile
                     (two banks, no write conflict),
             ONE exp ACT over [128,1024] (both heads) -> bf16,
             two AV matmuls accumulating [65,512] (ones column of V picks
                     up the softmax denominator).
           qkv(pair1) is emitted in small pieces between t-iterations of
           pair0's attention; projection chunks likewise ride inside
           pair1's attention.  The scalar engine (exp) is the critical
           resource; the PE fills its shadow.
  normalize per (p,cq): rowsum bcast via tiny K=1 matmul, reciprocal and
           multiply on DVE -> aoT bf16.
  proj: per s-tile: psum[128,512] = aoT[0].T @ wp[0] + aoT[1].T @ wp[1],
           copied to fp32 and DMA'd out (partial, host sums).
"""

import os
import sys

import numpy as np

try:
    import ml_dtypes
    BF16_NP = ml_dtypes.bfloat16
except ImportError:  # pragma: no cover
    BF16_NP = None

for _p in ("/opt/trn_rl_repo",):
    if os.path.isdir(_p) and _p not in sys.path:
        sys.path.append(_p)

import concourse.bass as bass  # noqa: E402
import concourse.mybir as mybir  # noqa: E402
import concourse.tile as tile  # noqa: E402
from concourse import bacc  # noqa: E402
from concourse.bass_utils import run_bass_kernel_spmd  # noqa: E402

B, S, D = 2, 2048, 1024
H, HD = 16, 64
N_CORES = 8
GROUP = 4           # cores per batch group
LH = H // GROUP     # local heads per core = 4 (2 pairs)
LHD = LH * HD       # 256 local head dims
FP32 = mybir.dt.float32
FP32R = mybir.dt.float32r
BF16 = mybir.dt.bfloat16

SQ = 512            # query chunk
N_CQ = S // SQ      # 4
N_SK = S // 128     # 16 key tiles
N_KT = D // 128     # 8 contraction tiles

_compiled = None
_ONES = np.ones((1, 64), dtype=np.float32)


def _build():
    nc = bacc.Bacc(
        "TRN2", target_bir_lowering=False, debug=False, num_devices=N_CORES
    )

    xT_d = nc.dram_tensor("xT", [D, S], BF16, kind="ExternalInput")
    wq_d = nc.dram_tensor("wq", [D, LHD], BF16, kind="ExternalInput")
    wk_d = nc.dram_tensor("wk", [D, LHD], BF16, kind="ExternalInput")
    wv_d = nc.dram_tensor("wv", [D, LHD], BF16, kind="ExternalInput")
    wp_d = nc.dram_tensor("wp", [LHD, D], BF16, kind="ExternalInput")
    ones_d = nc.dram_tensor("ones", [1, 64], FP32R, kind="ExternalInput")
    bq_d = nc.dram_tensor("bq", [LHD, 1], FP32, kind="ExternalInput")
    bk_d = nc.dram_tensor("bk", [LHD, 1], FP32, kind="ExternalInput")
    bv_d = nc.dram_tensor("bv", [128, LHD], FP32, kind="ExternalInput")
    out_d = nc.dram_tensor("out", [S, D], FP32, kind="ExternalOutput")

    with tile.TileContext(nc) as tc:
        import contextlib

        with contextlib.ExitStack() as stk:
            # ---- long-lived SBUF pools --------------------------------
            qk_pool = stk.enter_context(tc.tile_pool(name="qk", bufs=1))
            v_pool = stk.enter_context(tc.tile_pool(name="v", bufs=1))
            ao_pool = stk.enter_context(tc.tile_pool(name="ao", bufs=1))
            const_pool = stk.enter_context(tc.tile_pool(name="const", bufs=1))
            w_pool = stk.enter_context(tc.tile_pool(name="w", bufs=1))
            x_pool = stk.enter_context(tc.tile_pool(name="x", bufs=1))

            qT = [qk_pool.tile([128, S], BF16, name=f"qT{p}", tag=f"qT{p}")
                  for p in range(2)]
            kT = [qk_pool.tile([128, S], BF16, name=f"kT{p}", tag=f"kT{p}")
                  for p in range(2)]
            # vp[p][m]: [128 keys, 130] = head2p v | 1.0 | head2p+1 v | 1.0
            vp = [[v_pool.tile([128, 130], BF16, name=f"v{p}_{m}",
                               tag=f"v{p}_{m}") for m in range(N_SK)]
                  for p in range(2)]
            aoT = [ao_pool.tile([128, S], BF16, name=f"ao{p}", tag=f"ao{p}")
                   for p in range(2)]

            ones_t = const_pool.tile([1, 64], FP32R, tag="ones")
            nc.sync.dma_start(ones_t[:], ones_d.ap())
            bq_t = [const_pool.tile([128, 1], FP32, name=f"bq{p}",
                                    tag=f"bq{p}") for p in range(2)]
            bk_t = [const_pool.tile([128, 1], FP32, name=f"bk{p}",
                                    tag=f"bk{p}") for p in range(2)]
            bv_t = const_pool.tile([128, LHD], FP32, tag="bv")
            for p in range(2):
                psl = slice(p * 128, (p + 1) * 128)
                nc.sync.dma_start(bq_t[p][:], bq_d.ap()[psl, :])
                nc.sync.dma_start(bk_t[p][:], bk_d.ap()[psl, :])
            nc.sync.dma_start(bv_t[:], bv_d.ap())

            x_t = [x_pool.tile([128, S], BF16, name=f"x{k}", tag=f"x{k}")
                   for k in range(N_KT)]
            wq_t = [w_pool.tile([128, LHD], BF16, name=f"wq{k}", tag=f"wq{k}")
                    for k in range(N_KT)]
            wk_t = [w_pool.tile([128, LHD], BF16, name=f"wk{k}", tag=f"wk{k}")
                    for k in range(N_KT)]
            wv_t = [w_pool.tile([128, LHD], BF16, name=f"wv{k}", tag=f"wv{k}")
                    for k in range(N_KT)]
            wp_t = [w_pool.tile([128, D], BF16, name=f"wp{p}", tag=f"wp{p}")
                    for p in range(2)]

            # input DMA: x on sync queue, weights on scalar queue (scalar
            # engine is idle during phase A)
            for k in range(N_KT):
                sl = slice(k * 128, (k + 1) * 128)
                xeng = nc.sync if k % 2 == 0 else nc.gpsimd
                xeng.dma_start(x_t[k][:], xT_d.ap()[sl, :])
                nc.scalar.dma_start(wq_t[k][:], wq_d.ap()[sl, :])
                nc.scalar.dma_start(wk_t[k][:], wk_d.ap()[sl, :])
                nc.scalar.dma_start(wv_t[k][:], wv_d.ap()[sl, :])
            for p in range(2):
                nc.scalar.dma_start(wp_t[p][:],
                                    wp_d.ap()[p * 128:(p + 1) * 128, :])

            # ---- phase A: qk(pair0) k-major, v(pair0) m-major ---------
            with tc.tile_pool(name="psA", bufs=1, space="PSUM") as psA:
                ps_q = [psA.tile([128, SQ], FP32, name=f"psq{sc}",
                                 tag=f"psA{sc}") for sc in range(4)]
                ps_k = [psA.tile([128, SQ], FP32, name=f"psk{sc}",
                                 tag=f"psA{sc + 4}") for sc in range(4)]
                for k in range(N_KT):
                    for sc in range(4):
                        ssl = slice(sc * SQ, (sc + 1) * SQ)
                        nc.tensor.matmul(
                            ps_q[sc][:], wq_t[k][:, 0:128], x_t[k][:, ssl],
                            start=(k == 0), stop=(k == N_KT - 1),
                        )
                        nc.tensor.matmul(
                            ps_k[sc][:], wk_t[k][:, 0:128], x_t[k][:, ssl],
                            start=(k == 0), stop=(k == N_KT - 1),
                        )
                for sc in range(4):
                    ssl = slice(sc * SQ, (sc + 1) * SQ)
                    nc.vector.tensor_scalar(
                        qT[0][:, ssl], ps_q[sc][:], bq_t[0][:], None,
                        mybir.AluOpType.add,
                    )
                    nc.vector.tensor_scalar(
                        kT[0][:, ssl], ps_k[sc][:], bk_t[0][:], None,
                        mybir.AluOpType.add,
                    )

            with tc.tile_pool(name="psV", bufs=4, space="PSUM") as psV:
                for m in range(N_SK):
                    ps = psV.tile([128, LHD], FP32, tag="v")
                    for k in range(N_KT):
                        nc.tensor.matmul(
                            ps[:], x_t[k][:, m * 128:(m + 1) * 128],
                            wv_t[k][:],
                            start=(k == 0), stop=(k == N_KT - 1),
                        )
                    for p in range(2):
                        nc.vector.tensor_tensor(
                            vp[p][m][:, 0:64], ps[:, p * 128:p * 128 + 64],
                            bv_t[:, p * 128:p * 128 + 64],
                            mybir.AluOpType.add,
                        )
                        nc.vector.tensor_tensor(
                            vp[p][m][:, 65:129], ps[:, p * 128 + 64:p * 128 + 128],
                            bv_t[:, p * 128 + 64:p * 128 + 128],
                            mybir.AluOpType.add,
                        )
                        nc.vector.memset(vp[p][m][:, 64::65], 1.0)

            # ---- phase B pools ----------------------------------------
            sc_pool = stk.enter_context(
                tc.tile_pool(name="sc", bufs=2, space="PSUM"))
            acc_pool = stk.enter_context(
                tc.tile_pool(name="acc", bufs=1, space="PSUM"))
            misc_pool = stk.enter_context(
                tc.tile_pool(name="misc", bufs=2, space="PSUM"))
            p_pool = stk.enter_context(tc.tile_pool(name="pt", bufs=4))
            rr_pool = stk.enter_context(tc.tile_pool(name="rr", bufs=2))
            rc_pool = stk.enter_context(tc.tile_pool(name="rc", bufs=2))
            ost_pool = stk.enter_context(tc.tile_pool(name="ost", bufs=2))

            # ---- deferred emission units (PE filler work) -------------
            filler = []

            def qk1_chunk_parts(which, sc):
                # q/k pair-1 chunk: 8 accumulating MMs + DVE drain, split
                # into 2-MM pieces so injection granularity stays ~0.5us
                w_t, b_t, dstT = ((wq_t, bq_t[1], qT[1]) if which == "q"
                                  else (wk_t, bk_t[1], kT[1]))
                ssl = slice(sc * SQ, (sc + 1) * SQ)
                state = {}

                def piece(k0):
                    def emit():
                        if k0 == 0:
                            state["ps"] = misc_pool.tile(
                                [128, SQ], FP32,
                                name=f"mqk{which}{sc}", tag="m")
                        ps = state["ps"]
                        for k in (k0, k0 + 1):
                            nc.tensor.matmul(
                                ps[:], w_t[k][:, 128:256], x_t[k][:, ssl],
                                start=(k == 0), stop=(k == N_KT - 1),
                            )
                        if k0 == N_KT - 2:
                            nc.vector.tensor_scalar(
                                dstT[:, ssl], ps[:], b_t[:], None,
                                mybir.AluOpType.add,
                            )
                    return emit
                return [piece(k0) for k0 in range(0, N_KT, 2)]

            for sc in range(4):
                filler.extend(qk1_chunk_parts("q", sc))
                filler.extend(qk1_chunk_parts("k", sc))

            def proj_unit(m, nb):
                # partial projection for s-tile m, dout half nb
                msl = slice(m * 128, (m + 1) * 128)
                nsl = slice(nb * SQ, (nb + 1) * SQ)

                def emit():
                    ps = misc_pool.tile([128, SQ], FP32,
                                        name=f"mpj{m}_{nb}", tag="m")

                    for p in range(2):
                        nc.tensor.matmul(
                            ps[:], aoT[p][:, msl], wp_t[p][:, nsl],
                            start=(p == 0), stop=(p == 1),
                        )
                    ot = ost_pool.tile([128, SQ], FP32,
                                       name=f"ot{m}_{nb}", tag="ot")
                    nc.vector.tensor_copy(ot[:], ps[:])
                    nc.gpsimd.dma_start(out_d.ap()[msl, nsl], ot[:])
                return emit

            def inject(n):
                for _ in range(n):
                    if filler:
                        filler.pop(0)()

            # ---- phase B: attention -----------------------------------
            for p in range(2):
                for cq in range(N_CQ):
                    qsl = slice(cq * SQ, (cq + 1) * SQ)
                    acc_a = acc_pool.tile([65, SQ], FP32, tag="acca")
                    acc_b = acc_pool.tile([65, SQ], FP32, tag="accb")
                    for t in range(N_SK):
                        tsl = slice(t * 128, (t + 1) * 128)
                        sc_ab = sc_pool.tile([128, 2 * SQ], FP32, tag="sc")
                        nc.tensor.matmul(
                            sc_ab[:, 0:SQ], kT[p][0:64, tsl],
                            qT[p][0:64, qsl],
                            start=True, stop=True, tile_position=(0, 0),
                        )
                        nc.tensor.matmul(
                            sc_ab[:, SQ:2 * SQ], kT[p][64:128, tsl],
                            qT[p][64:128, qsl],
                            start=True, stop=True, tile_position=(64, 0),
                        )
                        pab = p_pool.tile([128, 2 * SQ], BF16, tag="pt")
                        nc.scalar.activation(
                            pab[:], sc_ab[:],
                            mybir.ActivationFunctionType.Exp, scale=0.125,
                        )
                        nc.tensor.matmul(
                            acc_a[:], vp[p][t][:, 0:65], pab[:, 0:SQ],
                            start=(t == 0), stop=(t == N_SK - 1),
                        )
                        nc.tensor.matmul(
                            acc_b[:], vp[p][t][:, 65:130], pab[:, SQ:2 * SQ],
                            start=(t == 0), stop=(t == N_SK - 1),
                        )
                        inject(2 if p == 0 else 1)
                    # normalize both heads of this (p, cq)
                    for acc, half in ((acc_a, 0), (acc_b, 1)):
                        rrow = rr_pool.tile([1, SQ], FP32R, tag="rrow")
                        nc.vector.tensor_copy(rrow[:], acc[64:65, :])
                        rbt = misc_pool.tile([128, SQ], FP32, tag="m",
                                             name=f"rb{p}{cq}{half}")
                        nc.tensor.matmul(
                            rbt[0:64, :], ones_t[0:1, 0:64], rrow[:],
                            start=True, stop=True,
                        )
                        rc = rc_pool.tile([64, SQ], FP32, tag="rc")
                        nc.vector.reciprocal_approx_fast(rc[:], rbt[0:64, :])
                        nc.vector.tensor_tensor(
                            aoT[p][64 * half:64 * half + 64, qsl],
                            acc[0:64, :], rc[:],
                            mybir.AluOpType.mult,
                        )
                    if p == 1:
                        # projection for the chunk finished one cq ago
                        # rides inside the next chunk's t-loop via filler
                        for m in range(cq * 4, cq * 4 + 4):
                            for nb in range(2):
                                filler.append(proj_unit(m, nb))
            # flush remaining filler (last chunk's projection)
            while filler:
                filler.pop(0)()

    nc.compile()
    return nc


def _get_program():
    global _compiled
    if _compiled is None:
        _compiled = _build()
    return _compiled


def _make_in_maps(x, w_qkv, b_qkv, w_proj, b_proj):
    x = np.asarray(x, dtype=np.float32)
    w_qkv = np.asarray(w_qkv, dtype=np.float32)
    b_qkv = np.asarray(b_qkv, dtype=np.float32)
    w_proj = np.asarray(w_proj, dtype=np.float32)

    in_maps = []
    for c in range(N_CORES):
        g, r = c // GROUP, c % GROUP
        xT = np.ascontiguousarray(x[g].T)
        csl = slice(r * LHD, (r + 1) * LHD)
        in_maps.append(
            {
                "xT": xT.astype(BF16_NP),
                "wq": w_qkv[:, 0 * D + r * LHD:0 * D + (r + 1) * LHD].astype(BF16_NP),
                "wk": w_qkv[:, 1 * D + r * LHD:1 * D + (r + 1) * LHD].astype(BF16_NP),
                "wv": w_qkv[:, 2 * D + r * LHD:2 * D + (r + 1) * LHD].astype(BF16_NP),
                "wp": np.ascontiguousarray(w_proj[csl, :]).astype(BF16_NP),
                "bq": np.ascontiguousarray(
                    b_qkv[0 * D + r * LHD:0 * D + (r + 1) * LHD].reshape(LHD, 1)),
                "bk": np.ascontiguousarray(
                    b_qkv[1 * D + r * LHD:1 * D + (r + 1) * LHD].reshape(LHD, 1)),
                "ones": _ONES,
                "bv": np.ascontiguousarray(
                    np.broadcast_to(
                        b_qkv[2 * D + r * LHD:2 * D + (r + 1) * LHD].reshape(1, LHD),
                        (128, LHD),
                    )
                ),
            }
        )
    return in_maps


def _assemble(results, b_proj):
    out = np.empty((B, S, D), dtype=np.float32)
    for g in range(B):
        acc = results[g * GROUP]["out"].copy()
        for r in range(1, GROUP):
            acc += results[g * GROUP + r]["out"]
        out[g] = acc + np.asarray(b_proj, dtype=np.float32).reshape(1, D)
    return out


def kernel(x, w_qkv, b_qkv, w_proj, b_proj):
    nc = _get_program()
    in_maps = _make_in_maps(x, w_qkv, b_qkv, w_proj, b_proj)
    res = run_bass_kernel_spmd(nc, in_maps, list(range(N_CORES)))
    return _assemble(res.results, b_proj)
